# revision 2
# baseline (speedup 1.0000x reference)
"""Enformer relative-position attention block on 8 Trainium2 NeuronCores. (v3: bf16 projections)

Sharding: core c handles batch b = c//4 and head pair hp = c%4 (heads 2hp,
2hp+1).  Wq/Wk/Wv/W_rel_k are sliced column-wise per head pair, Wo row-wise
(tensor parallel); x is sharded by batch.  Each core computes a partial
(n, dim) output; the host sums the 4 partials per batch and adds bo.

v2 restructure vs baseline:
  - x is transposed on the HOST (xT input) -> no PE transposes of x.
  - logits are computed TRANSPOSED: LT[j, i-block] = k_chunk^T-stationary
    matmul against qcT, so p^T (needed by the oT matmuls) is the natural
    exp output -- no per-block PE transposes of p.
  - the relative-shift logits rsh (bf16, [i, j] layout) are folded into
    the LT psum by transpose-matmuls (lhsT=rsh chunk, rhs=identity).
  - softmax row-sums come for free: a ones column appended to the v
    stationary makes row 64 of the po2 accumulator equal sum_j pT[j,i].
  - p / v / wwin / rsh are bf16 (halves the shift-DMA bytes; sim rel-err
    ~2.5e-3 vs the 2e-2 gate).

Device kernel phases (emission order):
  P2: q/k/v projections from xT supers (12 dim-chunks x 512 i-cols).
  P1: rel_k projection (posT @ W_rel_k slice) -- emitted after P2 so its
      DMAs overlap P2 compute and PE starts immediately on P2.
  P3: attention per (head, super): wwin window matmuls -> bf16 copy ->
      diagonal-AP DMA shift -> LT tiles (rshT transpose-matmuls + content
      matmul) -> Exp -> oT accumulation (v|ones stationary) -> normalize.
  P4: output projection oT^T @ Wo_slice, written per 128x512 block.
"""

import math

import numpy as np
import ml_dtypes

_bf16_np = ml_dtypes.bfloat16

import concourse.bass as bass
import concourse.mybir as mybir
from concourse import bacc
from concourse.masks import make_identity
from concourse.tile import TileContext
from concourse.bass_utils import run_bass_kernel_spmd

F32 = mybir.dt.float32
F32R = mybir.dt.float32r
BF16 = mybir.dt.bfloat16

HEADS, DIM, DK, DV, NRPF = 8, 1536, 64, 192, 192
N = 1536
NCH = N // 128           # 12 query chunks
DIMCH = DIM // 128       # 12 contraction chunks
TWO_N1 = 2 * N - 1       # 3071
POSW = TWO_N1 + 1        # padded to even width 3072
WIN = 1664               # padded rel-window width (cols >1662 unused)
WSTR = WIN - 1           # diagonal read row stride
NSUP = 3                 # supers of 512 rows
SCALE = DK ** -0.5
VW = 386                 # v tile width: [h0 v(192) | ones | h1 v(192) | ones]


def _r(ap):
    return ap.bitcast(F32R)


def _get_positional_embed_np(n, feature_size):
    """numpy mirror of the reference's jax positional embedding (float64)."""
    from scipy.special import gammaln, xlogy

    nb = feature_size // 6
    dist = np.arange(-n + 1, n, dtype=np.float64)
    ad = np.abs(dist)[:, None]

    max_range = math.log(n) / math.log(2.0)
    half_life = 2.0 ** np.linspace(3.0, max_range, nb)
    f_exp = np.exp(-math.log(2.0) / half_life[None, :] * ad)

    center_widths = 2.0 ** np.arange(1, nb + 1, dtype=np.float64) - 1.0
    f_cm = (center_widths[None, :] > ad).astype(np.float64)

    stddev = n / (2.0 * nb)
    start_mean = n / nb
    mean = np.linspace(start_mean, float(n), nb)[None, :]
    concentration = (mean / stddev) ** 2
    rate = mean / (stddev**2)
    log_unnorm = xlogy(concentration - 1.0, ad) - rate * ad
    log_norm = gammaln(concentration) - concentration * np.log(rate)
    probs = np.exp(log_unnorm - log_norm) + 1e-8
    f_g = probs / np.max(probs)

    emb = np.concatenate([f_exp, f_cm, f_g], axis=-1)
    return np.concatenate([emb, np.sign(dist)[:, None] * emb], axis=-1)


def build_nc(reps=1):
    nc = bacc.Bacc(None)

    xt_d = nc.declare_dram_parameter("xT_b", [DIM, N], BF16, isOutput=False)
    wq_d = nc.declare_dram_parameter("wq_s", [128, DIMCH * 128], BF16, isOutput=False)
    wk_d = nc.declare_dram_parameter("wk_s", [128, DIMCH * 128], BF16, isOutput=False)
    wv_d = nc.declare_dram_parameter("wv_s", [128, DIMCH * 384], BF16, isOutput=False)
    wrk_d = nc.declare_dram_parameter("wrk_s", [NRPF, 128], BF16, isOutput=False)
    post_d = nc.declare_dram_parameter("posT", [NRPF, POSW], BF16, isOutput=False)
    bc_d = nc.declare_dram_parameter("bc_s", [128], F32, isOutput=False)
    bp_d = nc.declare_dram_parameter("bp_s", [128], F32, isOutput=False)
    wo_d = nc.declare_dram_parameter("wo_s", [384, DIM], BF16, isOutput=False)
    out_d = nc.declare_dram_parameter("out_p", [N, DIM], BF16, isOutput=True)

    with TileContext(nc) as tc:
      for _rep in range(reps):
        with tc.tile_pool(name="const", bufs=1) as const, \
             tc.tile_pool(name="persist", bufs=1) as persist:
            ident_f = const.tile([128, 128], F32, name="identf", tag="identf")
            make_identity(nc, ident_f)
            ident_b = const.tile([128, 128], BF16, name="identb", tag="identb")
            nc.vector.tensor_copy(ident_b[:], ident_f[:])
            bc_t = const.tile([128, 1], F32, name="bc", tag="bc")
            bp_t = const.tile([128, 1], F32, name="bp", tag="bp")
            bpc_t = const.tile([128, 1], F32, name="bpc", tag="bpc")
            nc.sync.dma_start(out=bc_t[:], in_=bc_d.rearrange("(p o) -> p o", o=1))
            nc.sync.dma_start(out=bp_t[:], in_=bp_d.rearrange("(p o) -> p o", o=1))
            nc.vector.tensor_sub(bpc_t[:], bp_t[:], bc_t[:])

            # persistent per-head tensors
            qc_h = [persist.tile([64, N], F32R, name=f"qc{h}", tag=f"qc{h}") for h in range(2)]
            qp_h = [persist.tile([64, N], BF16, name=f"qp{h}", tag=f"qp{h}") for h in range(2)]
            k_h = [persist.tile([64, N], F32R, name=f"k{h}", tag=f"k{h}") for h in range(2)]
            relk_h = [persist.tile([64, POSW], BF16, name=f"rk{h}", tag=f"rk{h}") for h in range(2)]
            v_t = [persist.tile([128, VW], BF16, name=f"v{r}", tag=f"v{r}") for r in range(NCH)]
            o1_h = [persist.tile([128, N], BF16, name=f"o1{h}", tag=f"o1{h}") for h in range(2)]
            o2s = persist.tile([128, N], BF16, name="o2s", tag="o2s")
            for r in range(NCH):
                nc.vector.memset(v_t[r][:, 192:193], 1.0)
                nc.vector.memset(v_t[r][:, 385:386], 1.0)

            # attention SBUF pools opened early so the first two stepA calls
            # can run inside P2 (their shift DMAs hide under the k/v passes)
            _outer_cms = [tc.tile_pool(name="wwin", bufs=2),
                          tc.tile_pool(name="rsh", bufs=3),
                          tc.tile_pool(name="pt", bufs=3),
                          tc.tile_pool(name="small", bufs=2)]
            w_pool, rsh_pool, pt_pool, small = [p.__enter__() for p in _outer_cms]

            def stepA(h, s, pwpool, pwtag, engs=None):
                """wwin rel-window matmuls + bf16 copy + diagonal shift DMA."""
                rsh_t = []
                for g in range(4):
                    ci = 4 * s + g
                    i0 = 128 * ci
                    w0 = (N - 1) - i0 - 127
                    wwin = w_pool.tile([128, WIN], BF16, name="wwin", tag="wwin")
                    for pi, (c0, cw) in enumerate(
                            ((0, 512), (512, 512), (1024, 512), (1536, 128))):
                        pw = pwpool.tile([128, 512], F32, name="pw", tag=pwtag)
                        nc.tensor.matmul(
                            pw[:, :cw], qp_h[h][:, i0:i0 + 128],
                            relk_h[h][:, w0 + c0:w0 + c0 + cw],
                            start=True, stop=True)
                        e = (engs or "vvvs")[pi]
                        eng = nc.scalar.copy if e == "s" else nc.vector.tensor_copy
                        eng(wwin[:, c0:c0 + cw], pw[:, :cw])
                    rsh = rsh_pool.tile([128, N], BF16, name=f"rsh{g}", tag=f"rsh{g}")
                    diag = bass.AP(
                        tensor=wwin[:].tensor, offset=127,
                        ap=[[WSTR, 128], [1, N]])
                    nc.sync.dma_start(out=rsh[:], in_=diag)
                    rsh_t.append(rsh)
                return rsh_t

            # s-outer ordering: P4 for super s interleaves right after both
            # heads of super s complete.
            hs_list = [(h, s) for s in range(NSUP) for h in range(2)]
            rshq = []

            # ---- P2: chunk-major q matmuls paced with the xT DMA stream,
            #      P1 in the DMA-bound window, then k and v passes with the
            #      q activations and first stepA calls overlapped.
            with tc.tile_pool(name="xfull", bufs=1) as xp, \
                 tc.tile_pool(name="wvp", bufs=1) as wvp, \
                 tc.tile_pool(name="pos", bufs=1) as pos_pool, \
                 tc.tile_pool(name="wqk", bufs=1) as wqk:
                xf = xp.tile([128, DIMCH * N], BF16, name="xf", tag="xf")
                wv_t = wvp.tile([128, DIMCH * 384], BF16, name="wv", tag="wv")
                wq_t = wqk.tile([128, DIMCH * 128], BF16, name="wq", tag="wq")
                wk_t = wqk.tile([128, DIMCH * 128], BF16, name="wk", tag="wk")
                pos_a = pos_pool.tile([128, POSW], BF16, name="posA", tag="posA")
                pos_b = pos_pool.tile([64, POSW], BF16, name="posB", tag="posB")
                wrk_a = pos_pool.tile([128, 128], BF16, name="wrkA", tag="wrkA")
                wrk_b = pos_pool.tile([64, 128], BF16, name="wrkB", tag="wrkB")
                # hw-DGE loads all on SP, ordered by first consumption
                nc.sync.dma_start(out=wq_t[:], in_=wq_d[:, :])
                for rr in range(DIMCH):
                    nc.sync.dma_start(out=xf[:, N * rr:N * rr + N],
                                      in_=xt_d[128 * rr:128 * rr + 128, :])
                    if rr == 2:
                        nc.sync.dma_start(out=wrk_a[:], in_=wrk_d[0:128, :])
                        nc.sync.dma_start(out=wrk_b[:], in_=wrk_d[128:NRPF, :])
                        nc.sync.dma_start(out=pos_a[:], in_=post_d[0:128, :])
                        nc.sync.dma_start(out=pos_b[:], in_=post_d[128:NRPF, :])
                    if rr == 9:
                        nc.sync.dma_start(out=wk_t[:], in_=wk_d[:, :])
                nc.sync.dma_start(out=wv_t[:], in_=wv_d[:, :])

                with tc.tile_pool(name="ps_q", bufs=1, space="PSUM") as ps_q, \
                     tc.tile_pool(name="ps_rk", bufs=2, space="PSUM") as ps_rk, \
                     tc.tile_pool(name="ps_v", bufs=2, space="PSUM") as ps_v:
                    psq = [ps_q.tile([128, 512], F32, name=f"psq{s}", tag=f"psq{s}")
                           for s in range(NSUP)]
                    for rr in range(DIMCH):
                        for s in range(NSUP):
                            xs = xf[:, N * rr + 512 * s:N * rr + 512 * s + 512]
                            nc.tensor.matmul(psq[s][:], wq_t[:, 128 * rr:128 * rr + 128],
                                             xs, start=(rr == 0), stop=(rr == DIMCH - 1))
                        if rr == 4:
                            # P1: rel_k projection in the DMA-bound window
                            for cb in range(6):
                                c0 = 512 * cb
                                ps = ps_rk.tile([128, 512], F32, name="psrk", tag="psrk")
                                nc.tensor.matmul(ps[:], wrk_a[:], pos_a[:, c0:c0 + 512],
                                                 start=True, stop=False)
                                nc.tensor.matmul(ps[:], wrk_b[:], pos_b[:, c0:c0 + 512],
                                                 start=False, stop=True)
                                for h in range(2):
                                    eng = nc.scalar.copy if h == 0 else nc.vector.tensor_copy
                                    eng(relk_h[h][:, c0:c0 + 512], ps[64 * h:64 * h + 64, :])
                    # q activations: qc on Act; qp = qc + (bp-bc) on DVE
                    for s in range(NSUP):
                        cs = slice(512 * s, 512 * s + 512)
                        for h in range(2):
                            hs = slice(64 * h, 64 * h + 64)
                            nc.scalar.activation(qc_h[h][:, cs], psq[s][hs, :],
                                                 mybir.ActivationFunctionType.Identity,
                                                 bias=bc_t[hs, :], scale=SCALE)
                            nc.vector.tensor_scalar_add(qp_h[h][:, cs], qc_h[h][:, cs],
                                                        bpc_t[hs, :])
                    # first two stepA calls: wwin matmuls fill the activation
                    # wait; their shift DMAs hide under the k/v passes below
                    rshq.append(stepA(*hs_list[0], ps_rk, "psrk", engs="svsv"))
                    rshq.append(stepA(*hs_list[1], ps_rk, "psrk", engs="svsv"))
                    # k-pass reuses the q psum banks (per-tag versioning)
                    psk = [ps_q.tile([128, 512], F32, name=f"psq{s}", tag=f"psq{s}")
                           for s in range(NSUP)]
                    for rr in range(DIMCH):
                        for s in range(NSUP):
                            xs = xf[:, N * rr + 512 * s:N * rr + 512 * s + 512]
                            nc.tensor.matmul(psk[s][:], wk_t[:, 128 * rr:128 * rr + 128],
                                             xs, start=(rr == 0), stop=(rr == DIMCH - 1))
                    for s in range(NSUP):
                        cs = slice(512 * s, 512 * s + 512)
                        for h in range(2):
                            hs = slice(64 * h, 64 * h + 64)
                            nc.vector.tensor_copy(k_h[h][:, cs], psk[s][hs, :])

                    # ---- P2v: v projection (xf fully resident by now) ----
                    for ci in range(NCH):
                        psv = ps_v.tile([128, 384], F32, name="psv", tag="psv")
                        for rr in range(DIMCH):
                            nc.tensor.matmul(
                                psv[:],
                                xf[:, N * rr + 128 * ci:N * rr + 128 * ci + 128],
                                wv_t[:, 384 * rr:384 * rr + 384],
                                start=(rr == 0), stop=(rr == DIMCH - 1))
                        nc.vector.tensor_copy(v_t[ci][:, 0:192], psv[:, 0:192])
                        nc.scalar.copy(v_t[ci][:, 193:385], psv[:, 192:384])

            # ---------------- P3 + interleaved P4 ----------------
            with tc.tile_pool(name="ps_w", bufs=2, space="PSUM") as ps_w, \
                 tc.tile_pool(name="ps_c", bufs=2, space="PSUM") as ps_c, \
                 tc.tile_pool(name="ps_o", bufs=1, space="PSUM") as ps_o, \
                 tc.tile_pool(name="wo", bufs=1) as wo_pool, \
                 tc.tile_pool(name="osb", bufs=3) as osb_pool, \
                 tc.tile_pool(name="ps_f", bufs=2, space="PSUM") as ps_f:
                wo_t = [wo_pool.tile([128, DIM], BF16, name=f"wo{i}", tag=f"wo{i}")
                        for i in range(3)]
                for t, (r0, r1) in zip(wo_t, [(0, 128), (128, 256), (256, 384)]):
                    nc.sync.dma_start(out=t[:], in_=wo_d[r0:r1, :])
                o_pieces = [o1_h[0], o1_h[1], o2s]

                def stepB(h, s, rsh_t):
                    """LT tiles (content + rshT) -> exp -> oT accumulation."""
                    po1 = ps_o.tile([128, 512], F32, name="po1", tag="po1")
                    po2 = ps_o.tile([65, 512], F32, name="po2", tag="po2")
                    vo = 193 * h
                    prev = None

                    def po_emit(jb, pt):
                        nc.tensor.matmul(po1[:], v_t[jb][:, vo:vo + 128], pt[:],
                                         start=(jb == 0), stop=(jb == NCH - 1))
                        nc.tensor.matmul(po2[:], v_t[jb][:, vo + 128:vo + 193], pt[:],
                                         start=(jb == 0), stop=(jb == NCH - 1))

                    for jb in range(NCH):
                        j0 = 128 * jb
                        lt = ps_c.tile([128, 512], F32, name="lt", tag="lt")
                        nc.tensor.matmul(lt[:], _r(k_h[h][:, j0:j0 + 128]),
                                         _r(qc_h[h][:, 512 * s:512 * s + 512]),
                                         start=True, stop=False)
                        for g in range(4):
                            nc.tensor.matmul(lt[:, 128 * g:128 * g + 128],
                                             rsh_t[g][:, j0:j0 + 128], ident_b[:],
                                             start=False, stop=(g == 3))
                        pt = pt_pool.tile([128, 512], BF16, name="pt", tag="pt")
                        nc.scalar.activation(pt[:], lt[:],
                                             mybir.ActivationFunctionType.Exp)
                        if prev is not None:
                            po_emit(*prev)
                        prev = (jb, pt)
                    po_emit(*prev)
                    return po1, po2

                def stepC(h, s, po1, po2):
                    cs = slice(512 * s, 512 * s + 512)
                    rinv = small.tile([1, 512], F32, name="rinv", tag="rinv")
                    nc.vector.reciprocal(rinv[:], po2[64:65, :])
                    brow = small.tile([128, 512], F32, name="brow", tag="brow")
                    nc.gpsimd.partition_broadcast(brow[:], rinv[:])
                    nc.vector.tensor_mul(o1_h[h][:, cs], po1[:], brow[:])
                    nc.vector.tensor_mul(o2s[64 * h:64 * h + 64, cs], po2[0:64, :],
                                         brow[0:64, :])

                def stepP4(s):
                    for r in range(4 * s, 4 * s + 4):
                        i0 = 128 * r
                        osb = osb_pool.tile([128, DIM], BF16, name="osb", tag="osb")
                        for ob in range(3):
                            pf = ps_f.tile([128, 512], F32, name="pf", tag="pf")
                            for kc in range(3):
                                nc.tensor.matmul(
                                    pf[:], o_pieces[kc][:, i0:i0 + 128],
                                    wo_t[kc][:, 512 * ob:512 * ob + 512],
                                    start=(kc == 0), stop=(kc == 2))
                            eng = [nc.vector.tensor_copy, nc.scalar.copy,
                                   nc.vector.tensor_copy][ob]
                            eng(osb[:, 512 * ob:512 * ob + 512], pf[:])
                        nc.sync.dma_start(out=out_d[i0:i0 + 128, :], in_=osb[:])

                for i, (h, s) in enumerate(hs_list):
                    if i + 2 < len(hs_list):
                        rshq.append(stepA(*hs_list[i + 2], ps_w, "pw"))
                    rsh_cur = rshq.pop(0)
                    po1, po2 = stepB(h, s, rsh_cur)
                    stepC(h, s, po1, po2)
                    if h == 1:
                        stepP4(s)

            for p in reversed(_outer_cms):
                p.__exit__(None, None, None)

    nc.compile()
    return nc


_NC_CACHE = None


def _get_nc():
    global _NC_CACHE
    if _NC_CACHE is None:
        _NC_CACHE = build_nc()
    return _NC_CACHE


_POST_CACHE = None


def _get_posT():
    global _POST_CACHE
    if _POST_CACHE is None:
        p = _get_positional_embed_np(N, NRPF).T.astype(np.float32)
        _POST_CACHE = np.zeros((NRPF, POSW), np.float32)
        _POST_CACHE[:, :TWO_N1] = p
    return _POST_CACHE


def _wlayout(w, c):
    """[DIM, c] -> [128, DIMCH*c] bf16: chunk rr lands at cols [c*rr, c*rr+c)."""
    return np.ascontiguousarray(
        w.reshape(DIMCH, 128, c).transpose(1, 0, 2).reshape(128, DIMCH * c)
    ).astype(_bf16_np)


def make_in_maps(inputs):
    x = np.asarray(inputs["x"], np.float32)
    Wq = np.asarray(inputs["Wq"], np.float32)
    Wk = np.asarray(inputs["Wk"], np.float32)
    Wv = np.asarray(inputs["Wv"], np.float32)
    W_rel_k = np.asarray(inputs["W_rel_k"], np.float32)
    bc = np.asarray(inputs["rel_content_bias"], np.float32)[0, :, 0, :]  # (H, DK)
    bp = np.asarray(inputs["rel_pos_bias"], np.float32)[0, :, 0, :]
    Wo = np.asarray(inputs["Wo"], np.float32)
    posT_bf = _get_posT().astype(_bf16_np)
    xT = [np.ascontiguousarray(x[b].T).astype(_bf16_np) for b in range(2)]
    in_maps = []
    for core in range(8):
        b, hp = core // 4, core % 4
        in_maps.append({
            "xT_b": xT[b],
            "wq_s": _wlayout(Wq[:, 128 * hp:128 * hp + 128], 128),
            "wk_s": _wlayout(Wk[:, 128 * hp:128 * hp + 128], 128),
            "wv_s": _wlayout(Wv[:, 384 * hp:384 * hp + 384], 384),
            "wrk_s": np.ascontiguousarray(
                W_rel_k[:, 128 * hp:128 * hp + 128]).astype(_bf16_np),
            "posT": posT_bf,
            "bc_s": np.ascontiguousarray(bc[2 * hp:2 * hp + 2].reshape(128)),
            "bp_s": np.ascontiguousarray(bp[2 * hp:2 * hp + 2].reshape(128)),
            "wo_s": np.ascontiguousarray(np.concatenate([
                Wo[384 * hp:384 * hp + 128],
                Wo[384 * hp + 192:384 * hp + 320],
                Wo[384 * hp + 128:384 * hp + 192],
                Wo[384 * hp + 320:384 * hp + 384]], axis=0)).astype(_bf16_np),
        })
    return in_maps


def assemble(results, bo):
    out = np.zeros((2, N, DIM), np.float32)
    for core in range(8):
        out[core // 4] += results[core]["out_p"]
    out += np.asarray(bo, np.float32)
    return out


def kernel(x, Wq, Wk, Wv, W_rel_k, rel_content_bias, rel_pos_bias, Wo, bo):
    in_maps = make_in_maps(dict(
        x=x, Wq=Wq, Wk=Wk, Wv=Wv, W_rel_k=W_rel_k,
        rel_content_bias=rel_content_bias, rel_pos_bias=rel_pos_bias, Wo=Wo))
    nc = _get_nc()
    res = run_bass_kernel_spmd(nc, in_maps, list(range(8)))
    return assemble(res.results, bo)
